# revision 17
# baseline (speedup 1.0000x reference)
"""Trainium2 Bass kernel for nn_Attention (B=64, N=289, C=768, H=12).

Data-parallel over batch: 8 batches per NeuronCore x 8 cores, no collectives.

Per-core pipeline (all matmuls bf16 with f32 PSUM accumulation):
  1. qkv GEMM token-major.  Mean-centering of q/k is folded into the
     weights host-side (W' = W - per-head row mean, exact), so the GEMM
     directly emits zero-mean q/k and no mean stats are needed.  v heads
     are computed in the same per-block pass and scattered into per-batch
     token-major v65 tiles via SBUF->SBUF DMA (arbitrary partition shift).
  2. q/k head-wise layernorm reduces to one rsqrt scale: var = sumsq/64,
     rsqrt via DVE bit-trick + Newton, z = q * s.
  3. rope fused with the LN gain/bias via host-precomputed tables; even
     lanes on DVE, odd lanes on GPSIMD.  1/sqrt(D) folded into q tables.
  4. q/k -> feature-major strips via XBAR DMA transpose ([128,128] bf16
     SBUF->SBUF), split across the two HWDGE engines (k on SP, q on ACT)
     so the two transpose pipes run in parallel.
  5. scores computed transposed ST[k_tok, q_tok] (softmax without max
     subtraction -- LN-bounded logits), exp on ScalarE straight from PSUM.
  6. AV with a ones-column appended to V: OT[65, q] where row 64 carries
     the softmax sums; sum rows gathered via aligned ScalarE copies into
     stride-32 partitions + 3 partition-matched DMAs on the ACT queue.
     The normalize tail (reciprocal, broadcast, multiply) is deferred and
     emitted interleaved with the NEXT batch's scores so the PE never
     waits on it.
  7. proj GEMM feature-major + bias, bf16 out DMA as yT[768, 2312]; the
     host transposes back and casts to f32.
"""

import sys

if "/opt/trn_rl_repo" not in sys.path:
    sys.path.insert(0, "/opt/trn_rl_repo")

from contextlib import ExitStack

import ml_dtypes
import numpy as np

import concourse.bass as bass
import concourse.tile as tile
from concourse import bacc, mybir
from concourse.bass_utils import run_bass_kernel_spmd

F32 = mybir.dt.float32
BF16 = mybir.dt.bfloat16
I32 = mybir.dt.int32
BF = ml_dtypes.bfloat16
OP = mybir.AluOpType
AF = mybir.ActivationFunctionType

B, N, C, H = 64, 289, 768, 12
D = C // H  # 64
NCORES = 8
BPC = B // NCORES  # 8 batches per core
T = BPC * N  # 2312 tokens per core
NT = (T + 127) // 128  # 19 token blocks
TPAD = NT * 128  # 2432
EPS = 1e-5
CHW = 6 * D  # 384 columns per qkv chunk (6 heads; centering is in W)
MAGIC = 0x5F3759DF

_CACHE = {}


def _batch_blocks():
    out, r = [], 0
    while r < N:
        rows = min(128, N - r)
        out.append((r, rows))
        r += rows
    return out


def _block_pieces(i):
    """Split global token block i into per-(batch, kblock) pieces.

    Returns [(src_row, b, kb, dst_row, take)].
    """
    pieces = []
    g = 128 * i
    end = min(128 * (i + 1), T)
    while g < end:
        b = g // N
        l = g - N * b
        kb = l // 128
        row0 = l - 128 * kb
        take = min(end, N * (b + 1), N * b + 128 * (kb + 1)) - g
        pieces.append((g - 128 * i, b, kb, row0, take))
        g += take
    return pieces


def _build_program():
    nc = bacc.Bacc("TRN2", target_bir_lowering=False, debug=False,
                   num_devices=NCORES)

    xT = nc.dram_tensor("xT", [C, TPAD], BF16, kind="ExternalInput").ap()
    wqkvT = nc.dram_tensor("wqkvT", [C, 6 * CHW], BF16,
                           kind="ExternalInput").ap()
    wprojT = nc.dram_tensor("wprojT", [C, C], BF16, kind="ExternalInput").ap()
    pbias = nc.dram_tensor("pbias", [C], F32, kind="ExternalInput").ap()
    # fused rope+LN tables, order: qCT, qST, qB2, kCT, kST, kB2
    tabs = nc.dram_tensor("tabs", [6, TPAD, D], BF16,
                          kind="ExternalInput").ap()
    sel = nc.dram_tensor("sel", [12, C], BF16, kind="ExternalInput").ap()
    out = nc.dram_tensor("out", [C, T], BF16, kind="ExternalOutput").ap()

    with tile.TileContext(nc) as tc, ExitStack() as ctx:
        consts = ctx.enter_context(tc.tile_pool(name="consts", bufs=1))
        work = ctx.enter_context(tc.tile_pool(name="work", bufs=4))
        blk = ctx.enter_context(tc.tile_pool(name="blk", bufs=2))
        shal = ctx.enter_context(tc.tile_pool(name="shal", bufs=2))
        rope_scr = ctx.enter_context(tc.tile_pool(name="rope", bufs=2))
        strips = ctx.enter_context(tc.tile_pool(name="strips", bufs=1))
        vpool = ctx.enter_context(tc.tile_pool(name="vpool", bufs=9))
        ptpool = ctx.enter_context(tc.tile_pool(name="ptpool", bufs=8))
        otsbp = ctx.enter_context(tc.tile_pool(name="otsbp", bufs=13))
        otnpool = ctx.enter_context(tc.tile_pool(name="otnpool", bufs=13))
        bpool = ctx.enter_context(tc.tile_pool(name="bpool", bufs=2))
        mmps = ctx.enter_context(tc.tile_pool(name="mmps", bufs=2,
                                              space="PSUM"))
        scps = ctx.enter_context(tc.tile_pool(name="scps", bufs=2,
                                              space="PSUM"))
        otps = ctx.enter_context(tc.tile_pool(name="otps", bufs=2,
                                              space="PSUM"))

        # ---- persistent constants (bulk loads on the ACT queue) ----
        wq = []
        for c in range(6):
            t = consts.tile([128, 6 * CHW], BF16, tag=f"wq{c}")
            nc.sync.dma_start(t[:], wqkvT[c * 128:(c + 1) * 128, :])
            wq.append(t)
        wp = []
        for c in range(6):
            t = consts.tile([128, C], BF16, tag=f"wp{c}")
            nc.sync.dma_start(t[:], wprojT[c * 128:(c + 1) * 128, :])
            wp.append(t)
        pbias_t = consts.tile([128, 6], F32, tag="pbias")
        nc.sync.dma_start(pbias_t[:], pbias.rearrange("(a p) -> p a", p=128))
        sel_t = consts.tile([12, C], BF16, tag="sel")
        nc.sync.dma_start(sel_t[:], sel)
        magic_t = consts.tile([128, 24], I32, tag="magic")
        nc.vector.memset(magic_t[:], MAGIC)
        tab_t = []
        for k in range(6):
            t = consts.tile([128, NT, D], BF16, tag=f"tab{k}")
            nc.sync.dma_start(
                t[:], tabs[k].rearrange("(i p) d -> p i d", p=128))
            tab_t.append(t)

        # q/k feature-major strips, all head pairs in one tile: strip p
        # lives at [:, p, :].  One XBAR DMA transpose per (block, half)
        # fills all six strips (out[:, j, :] = in[:, 128j:128j+128].T).
        qT = strips.tile([128, 6, TPAD], BF16, tag="qT", name="qT")
        kT = strips.tile([128, 6, TPAD], BF16, tag="kT", name="kT")

        kblocks = _batch_blocks()
        v65 = {}  # (b, kb) -> tile [128, H, D+1]

        def get_v65(b, kb):
            if (b, kb) not in v65:
                t = vpool.tile([128, H, D + 1], BF16, tag="v65")
                nc.vector.memset(t[:, :, D:D + 1], 1.0)
                v65[(b, kb)] = t
            return v65[(b, kb)]

        # ---- phase A: qkv GEMM + LN + rope + DMA transpose, per 128-blk
        xg_tiles = {}

        def load_xg(i):
            t = blk.tile([128, 6, 128], BF16, tag="xg", name="xg")
            nc.sync.dma_start(
                t[:], xT.rearrange("(c p) t -> p c t", p=128)[
                    :, :, i * 128:(i + 1) * 128])
            xg_tiles[i] = t

        load_xg(0)

        def emit_tile(i):
            if i + 1 < NT:
                load_xg(i + 1)
            xg = xg_tiles.pop(i)

            sumsq = work.tile([128, 24], F32, tag="sumsq")
            qsb = blk.tile([128, H, D], BF16, tag="qsb")
            ksb = blk.tile([128, H, D], BF16, tag="ksb")
            halves = (qsb, ksb)
            # q/k GEMM chunks 0..3, PE-dense; evacuation trails on ScalarE
            for j in range(4):
                ps = mmps.tile([128, CHW], F32, tag="mm")
                for c in range(6):
                    nc.tensor.matmul(ps[:], lhsT=xg[:, c, :],
                                     rhs=wq[c][:, j * CHW:(j + 1) * CHW],
                                     start=(c == 0), stop=(c == 5))
                dst = halves[j // 2]
                jl = j % 2
                nc.scalar.copy(
                    dst[:, jl * 6:(jl + 1) * 6, :].rearrange(
                        "p h d -> p (h d)"), ps[:])
            # v GEMM chunks 4,5 -> vsb, then DMA-scatter to per-batch v65
            vsb = blk.tile([128, H, D], BF16, tag="vsb")
            for j in range(2):
                ps = mmps.tile([128, CHW], F32, tag="mm")
                for c in range(6):
                    nc.tensor.matmul(
                        ps[:], lhsT=xg[:, c, :],
                        rhs=wq[c][:, (4 + j) * CHW:(5 + j) * CHW],
                        start=(c == 0), stop=(c == 5))
                nc.vector.tensor_copy(
                    vsb[:, j * 6:(j + 1) * 6, :],
                    ps[:].rearrange("p (h d) -> p h d", d=D))
            for (src_row, b, kb, dst_row, take) in _block_pieces(i):
                vt = get_v65(b, kb)
                nc.sync.dma_start(
                    vt[dst_row:dst_row + take, :, :D],
                    vsb[src_row:src_row + take, :, :])

            # squares + per-head reduces (ScalarE + DVE)
            sq = shal.tile([128, H, D], BF16, tag="sq")
            for half in range(2):
                nc.scalar.activation(sq[:], halves[half][:], AF.Square)
                nc.vector.tensor_reduce(
                    out=sumsq[:, half * 12:(half + 1) * 12], in_=sq[:],
                    axis=mybir.AxisListType.X, op=OP.add)

            # one consolidated stats chain on [128, 24]:
            # s = rsqrt(sumsq/64 + eps)  (q/k are zero-mean by construction)
            vpe = shal.tile([128, 24], F32, tag="vpe")
            srs = work.tile([128, 24], F32, tag="srs")
            nt1 = shal.tile([128, 24], F32, tag="nt1")
            nc.vector.tensor_scalar(out=vpe[:], in0=sumsq[:],
                                    scalar1=1.0 / D, scalar2=float(EPS),
                                    op0=OP.mult, op1=OP.add)
            nc.vector.tensor_scalar(out=srs[:].bitcast(I32),
                                    in0=vpe[:].bitcast(I32), scalar1=1,
                                    scalar2=None,
                                    op0=OP.logical_shift_right)
            nc.vector.tensor_tensor(out=srs[:].bitcast(I32),
                                    in0=magic_t[:],
                                    in1=srs[:].bitcast(I32),
                                    op=OP.subtract)
            # Newton: y *= 1.5 - 0.5*x*y^2
            nc.vector.tensor_mul(nt1[:], srs[:], srs[:])
            nc.vector.tensor_mul(nt1[:], nt1[:], vpe[:])
            nc.vector.tensor_scalar(out=nt1[:], in0=nt1[:],
                                    scalar1=-0.5, scalar2=1.5,
                                    op0=OP.mult, op1=OP.add)
            nc.vector.tensor_mul(srs[:], srs[:], nt1[:])

            for half, (tb, rot_tag, dst, dq) in enumerate(
                    ((0, "qrot", qT, nc.sync), (3, "krot", kT, nc.sync))):
                hsb = halves[half]
                hsrs = srs[:, half * 12:(half + 1) * 12]
                z = blk.tile([128, H, D], BF16, tag="z" + rot_tag)
                # z = q * s with s broadcast along d (q is zero-mean)
                nc.vector.tensor_tensor(
                    out=z[:], in0=hsb[:],
                    in1=hsrs[:, :, None].broadcast_to([128, H, D]),
                    op=OP.mult)

                # rope (+ folded gain/bias), deinterleaved pair layout:
                # head cols [0:32] = even lanes, [32:64] = odd lanes
                rot = blk.tile([128, H, D], BF16, tag=rot_tag)
                zE, zO = z[:, :, 0:32], z[:, :, 32:64]
                CT = tab_t[tb][:, i, :]
                ST = tab_t[tb + 1][:, i, :]
                B2 = tab_t[tb + 2][:, i, :]

                def bc(ap):
                    return ap[:, None, :].broadcast_to([128, H, 32])

                a = rope_scr.tile([128, H, 32], BF16, tag="ra")
                b_ = rope_scr.tile([128, H, 32], BF16, tag="rb")
                nc.vector.tensor_mul(a[:], zE, bc(CT[:, 0:32]))
                nc.gpsimd.tensor_mul(b_[:], zO, bc(ST[:, 0:32]))
                nc.vector.tensor_sub(a[:], a[:], b_[:])
                nc.gpsimd.tensor_add(rot[:, :, 0:32], a[:], bc(B2[:, 0:32]))
                c_ = rope_scr.tile([128, H, 32], BF16, tag="rc")
                d_ = rope_scr.tile([128, H, 32], BF16, tag="rd")
                nc.gpsimd.tensor_mul(c_[:], zO, bc(CT[:, 32:64]))
                nc.gpsimd.tensor_mul(d_[:], zE, bc(ST[:, 32:64]))
                nc.gpsimd.tensor_add(c_[:], c_[:], d_[:])
                nc.gpsimd.tensor_add(rot[:, :, 32:64], c_[:],
                                     bc(B2[:, 32:64]))

                # one XBAR DMA transpose fills all 6 strips of this block
                dq.dma_start_transpose(
                    dst[:, :, i * 128:(i + 1) * 128],
                    rot[:].rearrange("p h d -> p (h d)"))

        # ---- phase B: per batch attention + proj ----
        pending_norm = []
        pending_proj = []

        def flush_norm():
            while pending_norm:
                otsb_, rinvb, b_ = pending_norm.pop(0)
                otn = []
                for p in range(6):
                    o = otnpool.tile([128, N], BF16, tag="otn")
                    # one matmul broadcasts both heads' reciprocal rows
                    # across the pair's 128 partitions
                    rb = otps.tile([128, 512], F32, tag="ot")
                    nc.tensor.matmul(rb[:, :N],
                                     lhsT=sel_t[:, p * 128:(p + 1) * 128],
                                     rhs=rinvb[:], start=True, stop=True)
                    nc.vector.tensor_mul(o[:], otsb_[p][:], rb[:, :N])
                    otn.append(o)
                pending_proj.append((otn, b_))

        def flush_proj(n):
            while len(pending_proj) > n:
                otn_, b_ = pending_proj.pop(0)
                ysb = bpool.tile([128, 6, N], BF16, tag="ysb", name="ysb")
                for co in range(6):
                    pp = scps.tile([128, 2, 512], F32, tag="sc", name="pp")
                    for cp in range(6):
                        nc.tensor.matmul(
                            pp[:, 0, :N],
                            lhsT=wp[cp][:, co * 128:(co + 1) * 128],
                            rhs=otn_[cp][:], start=(cp == 0), stop=(cp == 5))
                    nc.vector.tensor_tensor(
                        out=ysb[:, co, :], in0=pp[:, 0, :N],
                        in1=pbias_t[:, co:co + 1].broadcast_to([128, N]),
                        op=OP.add)
                nc.sync.dma_start(
                    out.rearrange("(co p) t -> p co t", p=128)[
                        :, :, b_ * N:(b_ + 1) * N], ysb[:])

        batch_state = {}

        def emit_strip(b, p):
            if p == 0:
                rbuf = bpool.tile([128, 3 * N], F32, tag="rowbuf",
                                  name="rowbuf")
                batch_state[b] = (rbuf, [None] * 6)
            rowbuf, otsb = batch_state[b]
            if True:
                pts = []
                for (r0, rows) in kblocks:
                    sc = scps.tile([128, 2, 512], F32, tag="sc")
                    kc = b * N + r0
                    for h in range(2):
                        nc.tensor.matmul(
                            sc[:rows, h, :N],
                            lhsT=kT[h * D:(h + 1) * D, p, kc:kc + rows],
                            rhs=qT[h * D:(h + 1) * D, p, b * N:(b + 1) * N],
                            start=True, stop=True,
                            tile_position=(h * D, 0))
                    pt = ptpool.tile([128, 2, N], BF16, tag="pt")
                    nc.scalar.activation(pt[:rows, :, :], sc[:rows, :, :N],
                                         AF.Exp)
                    pts.append(pt)
                osb = otsbp.tile([128, N], BF16, tag="otsb")
                for h in range(2):
                    hh = 2 * p + h
                    ot = otps.tile([128, 512], F32, tag="ot")
                    for ik, (r0, rows) in enumerate(kblocks):
                        nc.tensor.matmul(
                            ot[:D + 1, :N],
                            lhsT=v65[(b, ik)][:rows, hh, :],
                            rhs=pts[ik][:rows, h, :],
                            start=(ik == 0), stop=(ik == len(kblocks) - 1))
                    # sum row -> stride-32 partition, free chunk hh//4
                    nc.scalar.copy(
                        rowbuf[(hh % 4) * 32:(hh % 4) * 32 + 1,
                               (hh // 4) * N:(hh // 4 + 1) * N],
                        ot[D:D + 1, :N])
                    nc.scalar.copy(osb[h * D:(h + 1) * D, :], ot[:D, :N])
                otsb[p] = osb
                if p == 1:
                    # previous batch's normalize tail + proj, emitted here
                    # so its PE work hides the current gather/recip latency
                    flush_norm()
                    flush_proj(0)

        def emit_tail(b):
            rowbuf, otsb = batch_state.pop(b)
            # gather + reciprocal now, so rinvb is long ready by the time
            # the deferred rb/otn (flush_norm) hits the PE next batch
            sums_sb = bpool.tile([12, N], F32, tag="sums_sb")
            r4 = rowbuf[:].rearrange("(a b) (c n) -> a b c n", b=32, n=N)
            for c in range(3):
                nc.sync.dma_start(sums_sb[4 * c:4 * c + 4, :],
                                    r4[:, 0, c, :])
            rinv = bpool.tile([12, N], F32, tag="rinv")
            rinvb = bpool.tile([12, N], BF16, tag="rinvb")
            nc.vector.reciprocal_approx_fast(rinv[:], sums_sb[:])
            nc.vector.tensor_copy(rinvb[:], rinv[:])
            pending_norm.append((otsb, rinvb, b))

        # interleave phase B at STRIP granularity: ~3 strip-units between
        # token blocks so no engine queue builds a deep backlog and the
        # mmps evacuations clear quickly.
        units = [(b, u) for b in range(BPC) for u in range(7)]  # 6 strips+tail
        ucur = 0

        def unit_allowed(b, i):
            return (N * (b + 1) + 127) // 128 - 1 <= i - 3

        def emit_unit(b, u):
            if u < 6:
                emit_strip(b, u)
            else:
                emit_tail(b)

        for i in range(NT):
            emit_tile(i)
            target = max(0, (i - 4) * len(units) // (NT - 5))
            while ucur < min(target, len(units)) \
                    and unit_allowed(units[ucur][0], i):
                emit_unit(*units[ucur])
                ucur += 1
        while ucur < len(units):
            emit_unit(*units[ucur])
            ucur += 1
        flush_norm()
        flush_proj(0)

    nc.compile()
    return nc


def _host_tables(rope_tensor, qn_g, qn_b, kn_g, kn_b, P, L):
    """Fused rope+LN tables [6, TPAD, 64]: qCT,qST,qB2,kCT,kST,kB2."""
    n_img = N - P - L
    rt = np.asarray(rope_tensor, np.float64)
    cos = rt[:n_img, :, 0]
    sin = rt[:n_img, :, 1]
    c_full = np.ones((N, D // 2))
    s_full = np.zeros((N, D // 2))
    c_full[P:N - L] = cos
    s_full[P:N - L] = sin
    reps = TPAD // N + 2
    c_all = np.tile(c_full, (reps, 1))[:TPAD]
    s_all = np.tile(s_full, (reps, 1))[:TPAD]
    c_all[T:] = 1.0
    s_all[T:] = 0.0

    def mk(g, b):
        # deinterleaved layout: cols [0:32] = even lanes, [32:64] = odd
        g = np.asarray(g, np.float64)
        b = np.asarray(b, np.float64)
        ge, go = g[0::2], g[1::2]
        be, bo = b[0::2], b[1::2]
        CT = np.empty((TPAD, D))
        ST = np.empty((TPAD, D))
        B2 = np.empty((TPAD, D))
        CT[:, 0:32] = ge[None, :] * c_all
        CT[:, 32:64] = go[None, :] * c_all
        ST[:, 0:32] = go[None, :] * s_all
        ST[:, 32:64] = ge[None, :] * s_all
        B2[:, 0:32] = be[None, :] * c_all - bo[None, :] * s_all
        B2[:, 32:64] = bo[None, :] * c_all + be[None, :] * s_all
        return CT, ST, B2

    qsc = 1.0 / np.sqrt(D)
    qCT, qST, qB2 = mk(np.asarray(qn_g, np.float64) * qsc,
                       np.asarray(qn_b, np.float64) * qsc)
    kCT, kST, kB2 = mk(kn_g, kn_b)
    return np.stack([qCT, qST, qB2, kCT, kST, kB2]).astype(BF)


def _host_wqkv(qkv_w):
    """wqkvT [C, 6*CHW]: 6 chunks of 6 heads x 64 cols.

    q/k heads get mean-centering folded in (W' = W - per-head row mean --
    exact: the head-mean of q is linear in x) and their columns permuted
    to the deinterleaved rope-pair layout ([evens, odds]); dot products
    over d are invariant since q and k get the same permutation.  v heads
    stay in natural order.
    """
    wT = np.asarray(qkv_w, np.float64).T  # [C, 3C]
    deint = np.concatenate([np.arange(0, D, 2), np.arange(1, D, 2)])
    outw = np.empty((C, 6 * CHW), np.float64)
    for j in range(6):
        cols = wT[:, j * 384:(j + 1) * 384].reshape(C, 6, D)
        if j < 4:  # q, k: fold centering, then deinterleave
            cols = cols - cols.mean(axis=2, keepdims=True)
            cols = cols[:, :, deint]
        outw[:, j * CHW:(j + 1) * CHW] = cols.reshape(C, 384)
    return outw.astype(BF)


def _host_sel():
    s = np.zeros((12, C), np.float32)
    for k in range(12):
        s[k, k * D:(k + 1) * D] = 1.0
    return s.astype(BF)


def _make_in_maps(x, rope_tensor, qkv_w, proj_w, proj_b, qn_g, qn_b,
                  kn_g, kn_b, P, L):
    tabs = _host_tables(rope_tensor, qn_g, qn_b, kn_g, kn_b, P, L)
    wqkvT = _host_wqkv(qkv_w)
    wprojT = np.ascontiguousarray(
        np.asarray(proj_w, np.float32).T).astype(BF)
    pb = np.ascontiguousarray(np.asarray(proj_b, np.float32))
    sel = _host_sel()
    in_maps = []
    for core in range(NCORES):
        xc = x[core * BPC:(core + 1) * BPC].reshape(T, C)
        xTc = np.zeros((C, TPAD), BF)
        xTc[:, :T] = xc.T.astype(BF)
        in_maps.append({"xT": xTc, "wqkvT": wqkvT, "wprojT": wprojT,
                        "pbias": pb, "tabs": tabs, "sel": sel})
    return in_maps


def kernel(x, rope_tensor, qkv_w, proj_w, proj_b, qn_g, qn_b, kn_g, kn_b,
           num_prefix_tokens, num_latent_tokens, _spmd_kwargs=None):
    P = int(num_prefix_tokens)
    L = int(num_latent_tokens)
    x = np.asarray(x, np.float32)
    assert x.shape == (B, N, C), x.shape

    if "nc" not in _CACHE:
        _CACHE["nc"] = _build_program()
    nc = _CACHE["nc"]

    in_maps = _make_in_maps(x, rope_tensor, qkv_w, proj_w, proj_b,
                            qn_g, qn_b, kn_g, kn_b, P, L)
    res = run_bass_kernel_spmd(nc, in_maps, core_ids=list(range(NCORES)),
                               **(_spmd_kwargs or {}))
    outs = []
    for core in range(NCORES):
        yT = np.asarray(res.results[core]["out"], BF).astype(np.float32)
        outs.append(yT.T.reshape(BPC, N, C))
    full = np.concatenate(outs, axis=0).astype(np.float32)
    if _spmd_kwargs is not None:
        _CACHE["last_results"] = res
    return full
